# revision 11
# baseline (speedup 1.0000x reference)
"""Distributed single-head attention on 8 TRN2 NeuronCores.

Math (matches the reference):
    q = z @ Wq; k = z @ Wk; v = z @ Wv
    out = softmax(q k^T) * DK**-0.5 @ v

Sharding: z rows split 8 ways. Each core projects its own shard; K^T
(fp16) is all-gathered in four quarter-gathers (one per local j-tile)
and V (bf16) in two half-gathers, so the serialized CC stream starts
on the smallest possible first piece and the S phase starts ~10us
earlier. Flash-style row-block attention follows:
    S^T_j = K^T[:, j-tile] ^T-matmul Q^T           (fp16 operands, f32 PSUM)
    P_j   = exp(S^T_j - 40)                        (bf16, shift-invariant)
    rowsumT = ones^T @ P                           (PE)
    out   = (P^T-matmuls V) * (scale / rowsum)

Schedule notes (all measured on this part):
 - cross-core launch skew is ~25-30us and the first collective pays a
   ~20us pickup on top; the whole CC chain (~25-30us per 0.5MB gather,
   serialized) is co-critical with the PE, so gathers are split small
   and triggered as early as possible, in consumption order.
 - input HBM BW is ~250-300GB/s shared, so projections are t-outer
   across all 8 PSUM banks: the first matmul needs only the first
   (zT, Wk) chunk pair and compute paces the load stream.
 - ALL input loads go on the sync ring strictly in need-order (zT/Wk →
   Wv → Wq); staging DMAs get the otherwise-idle ACT ring; post-gather
   K^T loads ride sync (free by then); V loads ride SWDGE.
 - bounce buffers are laid out (p, m, n): staging writes and
   post-gather loads are one contiguous run per partition ((m, p, n)
   produced 512B scattered segments and ran far under line rate).
 - the AV phase accumulates in two passes (j%4 in {0,1} then {2,3})
   over all 8 PSUM banks, so the second V gather may land ~30us into
   AV without stalling it.

Precision: fp16 z/W/Q/K + f32 PSUM keeps logits to ~1e-2 abs err;
exp/V/AV in bf16. End-to-end rel err ~3e-3 (vs f32 reference).
"""

import numpy as np

SEQ, D, DK, DV = 4096, 1024, 1024, 1024
NCORES = 8
ROWS = SEQ // NCORES            # 512 rows per core
DT = D // 128                   # 8 contraction tiles (input dim)
MT = DK // 128                  # 8 dk tiles
ST = ROWS // 128                # 4 local seq tiles
JT = SEQ // 128                 # 32 global seq tiles
SHIFT = 40.0                    # constant logit shift (softmax-invariant)
SCALE = DK ** -0.5

KT_Q = DK * 128                 # elems in one K^T quarter (bf16-bitcast fp16)
V_H = (ROWS // 2) * DV          # elems in one V half (bf16)


def _build():
    import concourse.mybir as mybir
    import concourse.tile as tile
    from concourse import bacc

    F32 = mybir.dt.float32
    F16 = mybir.dt.float16
    BF16 = mybir.dt.bfloat16
    Exp = mybir.ActivationFunctionType.Exp

    nc = bacc.Bacc("TRN2", target_bir_lowering=False, debug=False, num_devices=NCORES)
    d_zT = nc.declare_dram_parameter("zT", [D, ROWS], F16, isOutput=False)
    d_wq = nc.declare_dram_parameter("Wq", [D, DK], F16, isOutput=False)
    d_wk = nc.declare_dram_parameter("Wk", [D, DK], F16, isOutput=False)
    d_wv = nc.declare_dram_parameter("Wv", [D, DV], F16, isOutput=False)
    d_out = nc.declare_dram_parameter("out", [ROWS, DV], F32, isOutput=True)

    rg = [list(range(NCORES))]

    with tile.TileContext(nc) as tc:
        with (
            tc.tile_pool(name="dram", bufs=1, space="DRAM") as dram,
            tc.tile_pool(name="qt", bufs=1) as qt_pool,
            tc.tile_pool(name="misc", bufs=1) as misc,
            tc.tile_pool(name="stage", bufs=2) as stage,
            tc.tile_pool(name="vg", bufs=1) as vg_pool,
            tc.tile_pool(name="expp", bufs=1) as expp,
            tc.tile_pool(name="outp", bufs=2) as outp,
        ):
            # ---- collective bounce buffers ----
            kt_in = [dram.tile([KT_Q], BF16, name=f"ktin{q}") for q in range(ST)]
            kt_out = [dram.tile([NCORES * KT_Q], BF16, addr_space="Shared",
                                name=f"ktout{q}") for q in range(ST)]
            va_in = dram.tile([V_H], BF16)
            va_out = dram.tile([NCORES * V_H], BF16, addr_space="Shared")
            vb_in = dram.tile([V_H], BF16)
            vb_out = dram.tile([NCORES * V_H], BF16, addr_space="Shared")

            # constants: full-width ones for the PE rowsum (M=128 runs at
            # standard N=512 rate; M=1 measured ~40% slower), exp bias
            ones128 = misc.tile([128, 128], BF16)
            nc.vector.memset(ones128[:], 1.0)
            bias_sb = misc.tile([128, 1], F32)
            nc.vector.memset(bias_sb[:], -SHIFT)
            # touch Exp once so the ACT table set loads before the S phase
            warm_sb = misc.tile([128, 1], F32)
            nc.scalar.activation(warm_sb[:], bias_sb[:], Exp,
                                 bias=bias_sb[:], scale=1.0)

            # ---------------- projection phase (scoped weights) ----------
            with (
                tc.tile_pool(name="wz", bufs=1) as wz,
                tc.tile_pool(name="ps_proj", bufs=8, space="PSUM") as ps_proj,
            ):
                # zT + Wk chunk pairs interleaved on the sync HWDGE ring so
                # the t-outer K projection starts after one pair lands
                zv = d_zT.rearrange("(t p) n -> p t n", p=128)
                wkv = d_wk.rearrange("(t p) m -> p t m", p=128)
                zT_sb, wk_sb = [], []
                for t in range(DT):
                    zt = wz.tile([128, ROWS], F16, name=f"zt{t}")
                    nc.sync.dma_start(zt[:], zv[:, t, :])
                    zT_sb.append(zt)
                    w = wz.tile([128, DK], F16, name=f"wk{t}")
                    nc.sync.dma_start(w[:], wkv[:, t, :])
                    wk_sb.append(w)
                # Wv/Wq behind zT/Wk on the SAME sync ring: strict FIFO
                # defers them so the K-critical 3MB gets full HBM bandwidth
                wv_sb, wq_sb = [], []
                for d_w, prefix, tiles in ((d_wv, "wv", wv_sb), (d_wq, "wq", wq_sb)):
                    wvw = d_w.rearrange("(t p) m -> p t m", p=128)
                    for t in range(DT):
                        w = wz.tile([128, DK], F16, name=f"{prefix}{t}")
                        nc.sync.dma_start(w[:], wvw[:, t, :])
                        tiles.append(w)

                # K^T shard [DK, ROWS] in four seq-quarter passes, t-outer
                # over all 8 PSUM banks; each quarter staged with ONE
                # contiguous dma on the ACT ring and gathered immediately
                for q in range(ST):
                    psk = [ps_proj.tile([128, 128], F32, tag="psproj",
                                        name=f"psk{q}{m}") for m in range(MT)]
                    for t in range(DT):
                        for m in range(MT):
                            nc.tensor.matmul(
                                psk[m][:], wk_sb[t][:, m * 128:(m + 1) * 128],
                                zT_sb[t][:, q * 128:(q + 1) * 128],
                                start=(t == 0), stop=(t == DT - 1))
                    kt_stage = stage.tile([128, MT, 128], F16, tag="ktstage")
                    for m in range(MT):
                        nc.vector.tensor_copy(kt_stage[:, m, :], psk[m][:])
                    nc.scalar.dma_start(
                        kt_in[q][:].rearrange("(p m n) -> p m n", p=128, n=128),
                        kt_stage[:].bitcast(BF16))
                    nc.gpsimd.collective_compute(
                        "AllGather", mybir.AluOpType.bypass, replica_groups=rg,
                        ins=[kt_in[q][:].opt()], outs=[kt_out[q][:].opt()])

                # V shard [ROWS, DV] bf16, t-outer; staged per seq-half
                # with one dma each, gathered as soon as staged
                psv = [ps_proj.tile([128, 512], F32, tag="psproj",
                                    name=f"psv{g}") for g in range(8)]
                for t in range(DT):
                    for s in range(ST):
                        for h in range(2):
                            nc.tensor.matmul(
                                psv[s * 2 + h][:],
                                zT_sb[t][:, s * 128:(s + 1) * 128],
                                wv_sb[t][:, h * 512:(h + 1) * 512],
                                start=(t == 0), stop=(t == DT - 1))
                for v_in, v_out, s0 in ((va_in, va_out, 0), (vb_in, vb_out, 2)):
                    v_stage = stage.tile([128, 2, DV], BF16, tag="vstage")
                    for s in (s0, s0 + 1):
                        for h in range(2):
                            nc.vector.tensor_copy(
                                v_stage[:, s - s0, h * 512:(h + 1) * 512],
                                psv[s * 2 + h][:])
                    nc.scalar.dma_start(
                        v_in[:].rearrange("(p s m) -> p s m", p=128, m=DV),
                        v_stage[:])
                    nc.gpsimd.collective_compute(
                        "AllGather", mybir.AluOpType.bypass, replica_groups=rg,
                        ins=[v_in[:].opt()], outs=[v_out[:].opt()])

                # Q^T: [DK, ROWS] fp16 resident, t-outer (tolerates the
                # staggered wq arrival behind wv on the sync ring)
                qt_sb = qt_pool.tile([128, MT, ROWS], F16)
                psq = [ps_proj.tile([128, 512], F32, tag="psproj",
                                    name=f"psq{m}") for m in range(MT)]
                for t in range(DT):
                    for m in range(MT):
                        nc.tensor.matmul(
                            psq[m][:], wq_sb[t][:, m * 128:(m + 1) * 128],
                            zT_sb[t][:],
                            start=(t == 0), stop=(t == DT - 1))
                for m in range(MT):
                    nc.vector.tensor_copy(qt_sb[:, m, :], psq[m][:])

            # V gathered: resident [128, JT, DV] bf16 (64KB/partition).
            # SWDGE loads, issued in (vA, vB) = consumption order.
            v_sb = vg_pool.tile([128, JT, DV], BF16)
            for v_out, soff in ((va_out, 0), (vb_out, 2)):
                for b in range(NCORES):
                    src = v_out[b * V_H:(b + 1) * V_H].rearrange(
                        "(p s m) -> p s m", p=128, m=DV)
                    nc.gpsimd.dma_start(
                        v_sb[:, b * ST + soff:b * ST + soff + 2, :], src)

            expS = expp.tile([128, JT, ROWS], BF16)

            # ---------------- S phase -------------------------------------
            # One stage per K^T quarter-gather: stage q computes the
            # j = 4b + q tiles for every rank b.
            with (
                tc.tile_pool(name="ktg", bufs=4) as ktg_pool,
                tc.tile_pool(name="ps_s", bufs=4, space="PSUM") as ps_s,
                tc.tile_pool(name="ps_rs", bufs=1, space="PSUM") as ps_rs,
            ):
                rs_ps = ps_rs.tile([128, 512], F32)
                n_rs = 0
                for q in range(ST):
                    for b in range(NCORES):
                        ktb = ktg_pool.tile([128, MT, 128], F16, tag="ktg")
                        nc.sync.dma_start(
                            ktb[:],
                            kt_out[q][b * KT_Q:(b + 1) * KT_Q].rearrange(
                                "(p m n) -> p m n", p=128, n=128).bitcast(F16))
                        j = b * ST + q
                        ps_S = ps_s.tile([128, 512], F32, tag="pss")
                        for t in range(MT):
                            nc.tensor.matmul(
                                ps_S[:], ktb[:, t, :], qt_sb[:, t, :],
                                start=(t == 0), stop=(t == MT - 1))
                        nc.scalar.activation(expS[:, j, :], ps_S[:], Exp,
                                             bias=bias_sb[:], scale=1.0)
                        nc.tensor.matmul(rs_ps[:], ones128[:], expS[:, j, :],
                                         start=(n_rs == 0),
                                         stop=(n_rs == JT - 1))
                        n_rs += 1

                # row-sum -> per-row reciprocal multipliers [128, ST]
                rs_sb = misc.tile([1, 512], F32)
                nc.vector.tensor_copy(rs_sb[:], rs_ps[0:1, :])
                rs_dram = dram.tile([1, 512], F32)
                nc.sync.dma_start(rs_dram[:], rs_sb[:])
                rs128 = misc.tile([128, ST], F32)
                nc.sync.dma_start(
                    rs128[:], rs_dram[0, :].rearrange("(r p) -> p r", p=128))
                mult_sb = misc.tile([128, ST], F32)
                nc.vector.reciprocal(mult_sb[:], rs128[:])
                nc.vector.tensor_scalar_mul(mult_sb[:], mult_sb[:], SCALE)

            # ---------------- AV phase ------------------------------------
            # Two accumulation passes over all 8 (h, r) PSUM banks:
            # pass 0 consumes j%4 in {0,1} (vA), pass 1 j%4 in {2,3} (vB) —
            # so vB may land up to ~30us into the AV phase without stalling.
            j_pass = ([j for j in range(JT) if j % 4 < 2],
                      [j for j in range(JT) if j % 4 >= 2])
            with tc.tile_pool(name="ps_o", bufs=8, space="PSUM") as ps_o:
                po = [ps_o.tile([128, 512], F32, tag="pso", name=f"po{g}")
                      for g in range(8)]
                for part in (0, 1):
                    for h in range(2):
                        for r in range(ST):
                            p = po[h * ST + r]
                            for idx, j in enumerate(j_pass[part]):
                                nc.tensor.matmul(
                                    p[:],
                                    expS[:, j, r * 128:(r + 1) * 128],
                                    v_sb[:, j, h * 512:(h + 1) * 512],
                                    start=(part == 0 and idx == 0),
                                    stop=(part == 1 and idx == len(j_pass[1]) - 1))
                            if part == 1:
                                o_sb = outp.tile([128, 512], F32, tag="osb")
                                nc.vector.tensor_scalar_mul(o_sb[:], p[:],
                                                            mult_sb[:, r:r + 1])
                                nc.sync.dma_start(
                                    d_out[r * 128:(r + 1) * 128,
                                          h * 512:(h + 1) * 512],
                                    o_sb[:])
    nc.compile()
    return nc


_BUILT = None


def kernel(z, Wq, Wk, Wv):
    global _BUILT
    from concourse.bass_utils import run_bass_kernel_spmd

    if _BUILT is None:
        _BUILT = _build()
    nc = _BUILT

    zT = np.ascontiguousarray(z.T).astype(np.float16)
    wq16 = Wq.astype(np.float16)
    wk16 = Wk.astype(np.float16)
    wv16 = Wv.astype(np.float16)
    in_maps = [
        {
            "zT": np.ascontiguousarray(zT[:, c * ROWS:(c + 1) * ROWS]),
            "Wq": wq16,
            "Wk": wk16,
            "Wv": wv16,
        }
        for c in range(NCORES)
    ]
    res = run_bass_kernel_spmd(nc, in_maps, list(range(NCORES)))
    out = np.concatenate([res.results[c]["out"] for c in range(NCORES)], axis=0)
    return out.astype(np.float32)


if __name__ == "__main__":
    rng = np.random.default_rng(0)
    z = rng.standard_normal((SEQ, D)).astype(np.float32)
    Wq = (0.02 * rng.standard_normal((D, DK))).astype(np.float32)
    Wk = (0.02 * rng.standard_normal((D, DK))).astype(np.float32)
    Wv = (0.02 * rng.standard_normal((D, DV))).astype(np.float32)
    out = kernel(z=z, Wq=Wq, Wk=Wk, Wv=Wv)
    print(out.shape, out.dtype)


# revision 16
# speedup vs baseline: 1.0087x; 1.0087x over previous
"""Distributed single-head attention on 8 TRN2 NeuronCores.

Math (matches the reference):
    q = z @ Wq; k = z @ Wk; v = z @ Wv
    out = softmax(q k^T) * DK**-0.5 @ v

Sharding: z rows split 8 ways. Each core projects its own shard; K^T
(fp16) is all-gathered in four quarter-gathers (one per local j-tile)
and V (bf16) in two half-gathers, so the serialized CC stream starts
on the smallest possible first piece and the S phase starts ~10us
earlier. Flash-style row-block attention follows:
    S^T_j = K^T[:, j-tile] ^T-matmul Q^T           (fp16 operands, f32 PSUM)
    P_j   = exp(S^T_j - 40)                        (bf16, shift-invariant)
    rowsumT = ones^T @ P                           (PE)
    out   = (P^T-matmuls V) * (scale / rowsum)

Schedule notes (all measured on this part):
 - cross-core launch skew is ~25-30us and the first collective pays a
   ~20us pickup on top; the whole CC chain (~25-30us per 0.5MB gather,
   serialized) is co-critical with the PE, so gathers are split small
   and triggered as early as possible, in consumption order.
 - input HBM BW is ~250-300GB/s shared, so projections are t-outer
   across all 8 PSUM banks: the first matmul needs only the first
   (zT, Wk) chunk pair and compute paces the load stream.
 - ALL input loads go on the sync ring strictly in need-order (zT/Wk →
   Wv → Wq); staging DMAs get the otherwise-idle ACT ring; post-gather
   K^T loads ride sync (free by then); V loads ride SWDGE.
 - bounce buffers are laid out (p, m, n): staging writes and
   post-gather loads are one contiguous run per partition ((m, p, n)
   produced 512B scattered segments and ran far under line rate).
 - the AV phase accumulates in two passes (j%4 in {0,1} then {2,3})
   over all 8 PSUM banks, so the second V gather may land ~30us into
   AV without stalling it.

Precision: fp16 z/W/Q/K + f32 PSUM keeps logits to ~1e-2 abs err;
exp/V/AV in bf16. End-to-end rel err ~3e-3 (vs f32 reference).
"""

import numpy as np

SEQ, D, DK, DV = 4096, 1024, 1024, 1024
NCORES = 8
ROWS = SEQ // NCORES            # 512 rows per core
DT = D // 128                   # 8 contraction tiles (input dim)
MT = DK // 128                  # 8 dk tiles
ST = ROWS // 128                # 4 local seq tiles
JT = SEQ // 128                 # 32 global seq tiles
SHIFT = 40.0                    # constant logit shift (softmax-invariant)
SCALE = DK ** -0.5

KT_Q = DK * 128                 # elems in one K^T quarter (bf16-bitcast fp16)
V_H = (ROWS // 2) * DV          # elems in one V half (bf16)


def _build():
    import concourse.mybir as mybir
    import concourse.tile as tile
    from concourse import bacc

    F32 = mybir.dt.float32
    F16 = mybir.dt.float16
    BF16 = mybir.dt.bfloat16
    Exp = mybir.ActivationFunctionType.Exp

    nc = bacc.Bacc("TRN2", target_bir_lowering=False, debug=False, num_devices=NCORES)
    d_zT = nc.declare_dram_parameter("zT", [D, ROWS], F16, isOutput=False)
    d_wq = nc.declare_dram_parameter("Wq", [D, DK], F16, isOutput=False)
    d_wk = nc.declare_dram_parameter("Wk", [D, DK], F16, isOutput=False)
    d_wv = nc.declare_dram_parameter("Wv", [D, DV], F16, isOutput=False)
    d_out = nc.declare_dram_parameter("out", [ROWS, DV], F32, isOutput=True)

    rg = [list(range(NCORES))]

    with tile.TileContext(nc) as tc:
        with (
            tc.tile_pool(name="dram", bufs=1, space="DRAM") as dram,
            tc.tile_pool(name="qt", bufs=1) as qt_pool,
            tc.tile_pool(name="misc", bufs=1) as misc,
            tc.tile_pool(name="stage", bufs=2) as stage,
            tc.tile_pool(name="vg", bufs=1) as vg_pool,
            tc.tile_pool(name="expp", bufs=1) as expp,
            tc.tile_pool(name="outp", bufs=2) as outp,
        ):
            # ---- collective bounce buffers ----
            # K^T goes out as two quarter-gathers (j=4b and j=4b+1 tiles,
            # earliest possible S start) plus one half-gather for the rest:
            # each gather costs ~22us on the serialized CC stream nearly
            # independent of size, so more splits would delay the tail.
            kt_in = [dram.tile([KT_Q], BF16, name="ktin0"),
                     dram.tile([KT_Q], BF16, name="ktin1"),
                     dram.tile([2 * KT_Q], BF16, name="ktin2")]
            kt_out = [dram.tile([NCORES * KT_Q], BF16, addr_space="Shared",
                                name="ktout0"),
                      dram.tile([NCORES * KT_Q], BF16, addr_space="Shared",
                                name="ktout1"),
                      dram.tile([NCORES * 2 * KT_Q], BF16, addr_space="Shared",
                                name="ktout2")]
            va_in = dram.tile([V_H], BF16)
            va_out = dram.tile([NCORES * V_H], BF16, addr_space="Shared")
            vb_in = dram.tile([V_H], BF16)
            vb_out = dram.tile([NCORES * V_H], BF16, addr_space="Shared")

            # constants: full-width ones for the PE rowsum (M=128 runs at
            # standard N=512 rate; M=1 measured ~40% slower), exp bias
            ones128 = misc.tile([128, 128], BF16)
            nc.vector.memset(ones128[:], 1.0)
            bias_sb = misc.tile([128, 1], F32)
            nc.vector.memset(bias_sb[:], -SHIFT)
            # touch Exp once so the ACT table set loads before the S phase
            warm_sb = misc.tile([128, 1], F32)
            nc.scalar.activation(warm_sb[:], bias_sb[:], Exp,
                                 bias=bias_sb[:], scale=1.0)

            # ---------------- projection phase (scoped weights) ----------
            with (
                tc.tile_pool(name="wz", bufs=1) as wz,
                tc.tile_pool(name="ps_proj", bufs=8, space="PSUM") as ps_proj,
            ):
                # zT + Wk chunk pairs interleaved on the sync HWDGE ring so
                # the t-outer K projection starts after one pair lands
                zv = d_zT.rearrange("(t p) n -> p t n", p=128)
                wkv = d_wk.rearrange("(t p) m -> p t m", p=128)
                zT_sb, wk_sb = [], []
                for t in range(DT):
                    zt = wz.tile([128, ROWS], F16, name=f"zt{t}")
                    nc.sync.dma_start(zt[:], zv[:, t, :])
                    zT_sb.append(zt)
                    w = wz.tile([128, DK], F16, name=f"wk{t}")
                    nc.sync.dma_start(w[:], wkv[:, t, :])
                    wk_sb.append(w)
                # Wv/Wq behind zT/Wk on the SAME sync ring: strict FIFO
                # defers them so the K-critical 3MB gets full HBM bandwidth
                wv_sb, wq_sb = [], []
                for d_w, prefix, tiles in ((d_wv, "wv", wv_sb), (d_wq, "wq", wq_sb)):
                    wvw = d_w.rearrange("(t p) m -> p t m", p=128)
                    for t in range(DT):
                        w = wz.tile([128, DK], F16, name=f"{prefix}{t}")
                        nc.sync.dma_start(w[:], wvw[:, t, :])
                        tiles.append(w)

                # K^T shard [DK, ROWS]: two seq-quarter passes then one
                # seq-half pass, t-outer over all 8 PSUM banks; each piece
                # staged with ONE contiguous dma on the ACT ring and
                # gathered immediately
                for piece, (c0, cols) in enumerate(((0, 128), (128, 128),
                                                    (256, 256))):
                    psk = [ps_proj.tile([128, cols], F32, tag="psproj",
                                        name=f"psk{piece}{m}") for m in range(MT)]
                    for t in range(DT):
                        for m in range(MT):
                            nc.tensor.matmul(
                                psk[m][:], wk_sb[t][:, m * 128:(m + 1) * 128],
                                zT_sb[t][:, c0:c0 + cols],
                                start=(t == 0), stop=(t == DT - 1))
                    kt_stage = stage.tile([128, MT, cols], F16, tag="ktstage")
                    for m in range(MT):
                        nc.vector.tensor_copy(kt_stage[:, m, :], psk[m][:])
                    nc.scalar.dma_start(
                        kt_in[piece][:].rearrange("(p m n) -> p m n",
                                                  p=128, n=cols),
                        kt_stage[:].bitcast(BF16))
                    nc.gpsimd.collective_compute(
                        "AllGather", mybir.AluOpType.bypass, replica_groups=rg,
                        ins=[kt_in[piece][:].opt()],
                        outs=[kt_out[piece][:].opt()])

                # V shard [ROWS, DV] bf16, t-outer; staged per seq-half
                # with one dma each, gathered as soon as staged
                psv = [ps_proj.tile([128, 512], F32, tag="psproj",
                                    name=f"psv{g}") for g in range(8)]
                for t in range(DT):
                    for s in range(ST):
                        for h in range(2):
                            nc.tensor.matmul(
                                psv[s * 2 + h][:],
                                zT_sb[t][:, s * 128:(s + 1) * 128],
                                wv_sb[t][:, h * 512:(h + 1) * 512],
                                start=(t == 0), stop=(t == DT - 1))
                for v_in, v_out, s0 in ((va_in, va_out, 0), (vb_in, vb_out, 2)):
                    v_stage = stage.tile([128, 2, DV], BF16, tag="vstage")
                    for s in (s0, s0 + 1):
                        for h in range(2):
                            nc.vector.tensor_copy(
                                v_stage[:, s - s0, h * 512:(h + 1) * 512],
                                psv[s * 2 + h][:])
                    nc.scalar.dma_start(
                        v_in[:].rearrange("(p s m) -> p s m", p=128, m=DV),
                        v_stage[:])
                    nc.gpsimd.collective_compute(
                        "AllGather", mybir.AluOpType.bypass, replica_groups=rg,
                        ins=[v_in[:].opt()], outs=[v_out[:].opt()])

                # Q^T: [DK, ROWS] fp16 resident, t-outer (tolerates the
                # staggered wq arrival behind wv on the sync ring)
                qt_sb = qt_pool.tile([128, MT, ROWS], F16)
                psq = [ps_proj.tile([128, 512], F32, tag="psproj",
                                    name=f"psq{m}") for m in range(MT)]
                for t in range(DT):
                    for m in range(MT):
                        nc.tensor.matmul(
                            psq[m][:], wq_sb[t][:, m * 128:(m + 1) * 128],
                            zT_sb[t][:],
                            start=(t == 0), stop=(t == DT - 1))
                for m in range(MT):
                    nc.vector.tensor_copy(qt_sb[:, m, :], psq[m][:])

            # V gathered: resident [128, JT, DV] bf16 (64KB/partition).
            # SWDGE loads, issued in (vA, vB) = consumption order.
            v_sb = vg_pool.tile([128, JT, DV], BF16)
            for v_out, soff in ((va_out, 0), (vb_out, 2)):
                for b in range(NCORES):
                    src = v_out[b * V_H:(b + 1) * V_H].rearrange(
                        "(p s m) -> p s m", p=128, m=DV)
                    nc.gpsimd.dma_start(
                        v_sb[:, b * ST + soff:b * ST + soff + 2, :], src)

            expS = expp.tile([128, JT, ROWS], BF16)

            # ---------------- S phase -------------------------------------
            # One stage per K^T quarter-gather: stage q computes the
            # j = 4b + q tiles for every rank b.
            with (
                tc.tile_pool(name="ktg", bufs=4) as ktg_pool,
                tc.tile_pool(name="ps_s", bufs=4, space="PSUM") as ps_s,
                tc.tile_pool(name="ps_rs", bufs=1, space="PSUM") as ps_rs,
            ):
                rs_ps = ps_rs.tile([128, 512], F32)
                n_rs = 0
                for piece, (q0, nq) in enumerate(((0, 1), (1, 1), (2, 2))):
                    ncols = nq * 128
                    for b in range(NCORES):
                        ktb = ktg_pool.tile([128, MT, ncols], F16,
                                            tag=f"ktg{nq}")
                        nc.sync.dma_start(
                            ktb[:],
                            kt_out[piece][b * ncols * DK:
                                          (b + 1) * ncols * DK].rearrange(
                                "(p m n) -> p m n", p=128,
                                n=ncols).bitcast(F16))
                        for qq in range(nq):
                            j = b * ST + q0 + qq
                            ps_S = ps_s.tile([128, 512], F32, tag="pss")
                            for t in range(MT):
                                nc.tensor.matmul(
                                    ps_S[:],
                                    ktb[:, t, qq * 128:(qq + 1) * 128],
                                    qt_sb[:, t, :],
                                    start=(t == 0), stop=(t == MT - 1))
                            nc.scalar.activation(expS[:, j, :], ps_S[:], Exp,
                                                 bias=bias_sb[:], scale=1.0)
                            nc.tensor.matmul(rs_ps[:], ones128[:],
                                             expS[:, j, :],
                                             start=(n_rs == 0),
                                             stop=(n_rs == JT - 1))
                            n_rs += 1

                # row-sum -> per-row reciprocal multipliers [128, ST]
                rs_sb = misc.tile([1, 512], F32)
                nc.vector.tensor_copy(rs_sb[:], rs_ps[0:1, :])
                rs_dram = dram.tile([1, 512], F32)
                nc.sync.dma_start(rs_dram[:], rs_sb[:])
                rs128 = misc.tile([128, ST], F32)
                nc.sync.dma_start(
                    rs128[:], rs_dram[0, :].rearrange("(r p) -> p r", p=128))
                mult_sb = misc.tile([128, ST], F32)
                nc.vector.reciprocal(mult_sb[:], rs128[:])
                nc.vector.tensor_scalar_mul(mult_sb[:], mult_sb[:], SCALE)

            # ---------------- AV phase ------------------------------------
            # Two accumulation passes over all 8 (h, r) PSUM banks:
            # pass 0 consumes j%4 in {0,1} (vA), pass 1 j%4 in {2,3} (vB) —
            # so vB may land up to ~30us into the AV phase without stalling.
            j_pass = ([j for j in range(JT) if j % 4 < 2],
                      [j for j in range(JT) if j % 4 >= 2])
            with tc.tile_pool(name="ps_o", bufs=8, space="PSUM") as ps_o:
                po = [ps_o.tile([128, 512], F32, tag="pso", name=f"po{g}")
                      for g in range(8)]
                for part in (0, 1):
                    for h in range(2):
                        for r in range(ST):
                            p = po[h * ST + r]
                            for idx, j in enumerate(j_pass[part]):
                                nc.tensor.matmul(
                                    p[:],
                                    expS[:, j, r * 128:(r + 1) * 128],
                                    v_sb[:, j, h * 512:(h + 1) * 512],
                                    start=(part == 0 and idx == 0),
                                    stop=(part == 1 and idx == len(j_pass[1]) - 1))
                            if part == 1:
                                o_sb = outp.tile([128, 512], F32, tag="osb")
                                nc.vector.tensor_scalar_mul(o_sb[:], p[:],
                                                            mult_sb[:, r:r + 1])
                                nc.sync.dma_start(
                                    d_out[r * 128:(r + 1) * 128,
                                          h * 512:(h + 1) * 512],
                                    o_sb[:])
    nc.compile()
    return nc


_BUILT = None


def kernel(z, Wq, Wk, Wv):
    global _BUILT
    from concourse.bass_utils import run_bass_kernel_spmd

    if _BUILT is None:
        _BUILT = _build()
    nc = _BUILT

    zT = np.ascontiguousarray(z.T).astype(np.float16)
    wq16 = Wq.astype(np.float16)
    wk16 = Wk.astype(np.float16)
    wv16 = Wv.astype(np.float16)
    in_maps = [
        {
            "zT": np.ascontiguousarray(zT[:, c * ROWS:(c + 1) * ROWS]),
            "Wq": wq16,
            "Wk": wk16,
            "Wv": wv16,
        }
        for c in range(NCORES)
    ]
    res = run_bass_kernel_spmd(nc, in_maps, list(range(NCORES)))
    out = np.concatenate([res.results[c]["out"] for c in range(NCORES)], axis=0)
    return out.astype(np.float32)


if __name__ == "__main__":
    rng = np.random.default_rng(0)
    z = rng.standard_normal((SEQ, D)).astype(np.float32)
    Wq = (0.02 * rng.standard_normal((D, DK))).astype(np.float32)
    Wk = (0.02 * rng.standard_normal((D, DK))).astype(np.float32)
    Wv = (0.02 * rng.standard_normal((D, DV))).astype(np.float32)
    out = kernel(z=z, Wq=Wq, Wk=Wk, Wv=Wv)
    print(out.shape, out.dtype)


# revision 18
# speedup vs baseline: 1.0484x; 1.0394x over previous
"""Distributed single-head attention on 8 TRN2 NeuronCores.

Math (matches the reference):
    q = z @ Wq; k = z @ Wk; v = z @ Wv
    out = softmax(q k^T) * DK**-0.5 @ v

Sharding: z rows split 8 ways. Each core projects its own shard; K^T
(fp16) and V (bf16) shards are all-gathered in four async halves
(kt1, kt2, vA, vB — split along local seq) so each gather triggers as
early as possible. Flash-style row-block attention follows:
    S^T_j = K^T[:, j-tile] ^T-matmul Q^T           (fp16 operands, f32 PSUM)
    P_j   = exp(S^T_j - 40)                        (bf16, shift-invariant)
    rowsumT = ones^T @ P                           (PE)
    out   = (P^T-matmuls V) * (scale / rowsum)

Schedule notes (all measured on this part):
 - input HBM BW is only ~190GB/s (LNC1 port sharing), so projections are
   t-outer across all 8 PSUM banks: the first matmul needs only the first
   (zT, Wk) chunk pair, and compute paces the load stream.
 - ALL input loads go on the sync ring strictly in need-order (zT/Wk →
   Wv → Wq): any concurrent ring halves the zT/Wk arrival rate and delays
   the first gather. Staging DMAs get the otherwise-idle ACT ring;
   post-gather K^T loads ride sync (free by then); V loads ride SWDGE.
 - bounce buffers are laid out (p, m, n) so staging writes and
   post-gather loads are one contiguous 4KB segment per partition
   (the (m, p, n) layout produced 512B scattered segments and ran ~5x
   slower than line rate).
 - the AV phase accumulates in two passes (j%4 in {0,1} then {2,3}) over
   all 8 PSUM banks, so the second V gather may land ~30us into AV.

Precision: fp16 z/W/Q/K + f32 PSUM keeps logits to ~1e-2 abs err;
exp/V/AV in bf16. End-to-end rel err ~3e-3 (vs f32 reference).
"""

import numpy as np

SEQ, D, DK, DV = 4096, 1024, 1024, 1024
NCORES = 8
ROWS = SEQ // NCORES            # 512 rows per core
DT = D // 128                   # 8 contraction tiles (input dim)
MT = DK // 128                  # 8 dk tiles
ST = ROWS // 128                # 4 local seq tiles
JT = SEQ // 128                 # 32 global seq tiles
HN = ROWS // 2                  # 256 = half the local rows
SHIFT = 40.0                    # constant logit shift (softmax-invariant)
SCALE = DK ** -0.5

KT_H = DK * HN                  # elems in one K^T half (bf16-bitcast fp16)
V_H = HN * DV                   # elems in one V half (bf16)


def _build():
    import concourse.mybir as mybir
    import concourse.tile as tile
    from concourse import bacc

    F32 = mybir.dt.float32
    F16 = mybir.dt.float16
    BF16 = mybir.dt.bfloat16
    Exp = mybir.ActivationFunctionType.Exp

    nc = bacc.Bacc("TRN2", target_bir_lowering=False, debug=False, num_devices=NCORES)
    d_zT = nc.declare_dram_parameter("zT", [D, ROWS], F16, isOutput=False)
    d_wq = nc.declare_dram_parameter("Wq", [D, DK], F16, isOutput=False)
    d_wk = nc.declare_dram_parameter("Wk", [D, DK], F16, isOutput=False)
    d_wv = nc.declare_dram_parameter("Wv", [D, DV], F16, isOutput=False)
    d_out = nc.declare_dram_parameter("out", [ROWS, DV], F32, isOutput=True)

    rg = [list(range(NCORES))]

    with tile.TileContext(nc) as tc:
        with (
            tc.tile_pool(name="dram", bufs=1, space="DRAM") as dram,
            tc.tile_pool(name="qt", bufs=1) as qt_pool,
            tc.tile_pool(name="misc", bufs=1) as misc,
            tc.tile_pool(name="stage", bufs=2) as stage,
            tc.tile_pool(name="vg", bufs=1) as vg_pool,
            tc.tile_pool(name="expp", bufs=1) as expp,
            tc.tile_pool(name="outp", bufs=2) as outp,
        ):
            # ---- collective bounce buffers (per local-seq half) ----
            kt1_in = dram.tile([KT_H], BF16)
            kt1_out = dram.tile([NCORES * KT_H], BF16, addr_space="Shared")
            kt2_in = dram.tile([KT_H], BF16)
            kt2_out = dram.tile([NCORES * KT_H], BF16, addr_space="Shared")
            va_in = dram.tile([V_H], BF16)
            va_out = dram.tile([NCORES * V_H], BF16, addr_space="Shared")
            vb_in = dram.tile([V_H], BF16)
            vb_out = dram.tile([NCORES * V_H], BF16, addr_space="Shared")

            # constants: full-width ones for the PE rowsum (M=128 runs at
            # standard N=512 rate; M=1 measured ~40% slower), exp bias
            ones128 = misc.tile([128, 128], BF16)
            nc.vector.memset(ones128[:], 1.0)
            bias_sb = misc.tile([128, 1], F32)
            nc.vector.memset(bias_sb[:], -SHIFT)
            # touch Exp once so the ACT table set loads before the S phase
            warm_sb = misc.tile([128, 1], F32)
            nc.scalar.activation(warm_sb[:], bias_sb[:], Exp,
                                 bias=bias_sb[:], scale=1.0)

            # ---------------- projection phase (scoped weights) ----------
            with (
                tc.tile_pool(name="wz", bufs=1) as wz,
                tc.tile_pool(name="ps_proj", bufs=8, space="PSUM") as ps_proj,
            ):
                # zT + Wk chunk pairs interleaved on the sync HWDGE ring so
                # the t-outer K projection starts after one pair lands
                zv = d_zT.rearrange("(t p) n -> p t n", p=128)
                wkv = d_wk.rearrange("(t p) m -> p t m", p=128)
                zT_sb, wk_sb = [], []
                for t in range(DT):
                    zt = wz.tile([128, ROWS], F16, name=f"zt{t}")
                    nc.sync.dma_start(zt[:], zv[:, t, :])
                    zT_sb.append(zt)
                    w = wz.tile([128, DK], F16, name=f"wk{t}")
                    nc.sync.dma_start(w[:], wkv[:, t, :])
                    wk_sb.append(w)
                # Wv/Wq behind zT/Wk on the SAME sync ring: strict FIFO
                # defers them so the K-critical 3MB gets full HBM bandwidth
                wv_sb, wq_sb = [], []
                for d_w, prefix, tiles in ((d_wv, "wv", wv_sb), (d_wq, "wq", wq_sb)):
                    wvw = d_w.rearrange("(t p) m -> p t m", p=128)
                    for t in range(DT):
                        w = wz.tile([128, DK], F16, name=f"{prefix}{t}")
                        nc.sync.dma_start(w[:], wvw[:, t, :])
                        tiles.append(w)

                # K^T shard [DK, ROWS], two seq-halves; t-outer over all 8
                # PSUM banks; each half staged to DRAM with ONE dma
                for half, kt_in, kt_out in ((0, kt1_in, kt1_out),
                                            (1, kt2_in, kt2_out)):
                    psk = [ps_proj.tile([128, HN], F32, tag="psproj",
                                        name=f"psk{half}{m}") for m in range(MT)]
                    for t in range(DT):
                        for m in range(MT):
                            nc.tensor.matmul(
                                psk[m][:], wk_sb[t][:, m * 128:(m + 1) * 128],
                                zT_sb[t][:, half * HN:(half + 1) * HN],
                                start=(t == 0), stop=(t == DT - 1))
                    kt_stage = stage.tile([128, MT, HN], F16, tag="ktstage")
                    for m in range(MT):
                        nc.vector.tensor_copy(kt_stage[:, m, :], psk[m][:])
                    nc.scalar.dma_start(
                        kt_in[:].rearrange("(p m n) -> p m n", p=128, n=HN),
                        kt_stage[:].bitcast(BF16))
                    nc.gpsimd.collective_compute(
                        "AllGather", mybir.AluOpType.bypass, replica_groups=rg,
                        ins=[kt_in[:].opt()], outs=[kt_out[:].opt()])

                # V shard [ROWS, DV] bf16, t-outer; staged per seq-half
                # with one dma each, gathered as soon as staged
                psv = [ps_proj.tile([128, 512], F32, tag="psproj",
                                    name=f"psv{g}") for g in range(8)]
                for t in range(DT):
                    for s in range(ST):
                        for h in range(2):
                            nc.tensor.matmul(
                                psv[s * 2 + h][:],
                                zT_sb[t][:, s * 128:(s + 1) * 128],
                                wv_sb[t][:, h * 512:(h + 1) * 512],
                                start=(t == 0), stop=(t == DT - 1))
                for v_in, v_out, s0 in ((va_in, va_out, 0), (vb_in, vb_out, 2)):
                    v_stage = stage.tile([128, 2, DV], BF16, tag="vstage")
                    for s in (s0, s0 + 1):
                        for h in range(2):
                            nc.vector.tensor_copy(
                                v_stage[:, s - s0, h * 512:(h + 1) * 512],
                                psv[s * 2 + h][:])
                    nc.scalar.dma_start(
                        v_in[:].rearrange("(p s m) -> p s m", p=128, m=DV),
                        v_stage[:])
                    nc.gpsimd.collective_compute(
                        "AllGather", mybir.AluOpType.bypass, replica_groups=rg,
                        ins=[v_in[:].opt()], outs=[v_out[:].opt()])

                # Q^T: [DK, ROWS] fp16 resident, t-outer (tolerates the
                # staggered wq arrival on the ACT ring)
                qt_sb = qt_pool.tile([128, MT, ROWS], F16)
                psq = [ps_proj.tile([128, 512], F32, tag="psproj",
                                    name=f"psq{m}") for m in range(MT)]
                for t in range(DT):
                    for m in range(MT):
                        nc.tensor.matmul(
                            psq[m][:], wq_sb[t][:, m * 128:(m + 1) * 128],
                            zT_sb[t][:],
                            start=(t == 0), stop=(t == DT - 1))
                for m in range(MT):
                    nc.vector.tensor_copy(qt_sb[:, m, :], psq[m][:])

            # V gathered: resident [128, JT, DV] bf16 (64KB/partition).
            # SWDGE loads, issued in (vA, vB) = consumption order.
            v_sb = vg_pool.tile([128, JT, DV], BF16)
            for v_out, soff in ((va_out, 0), (vb_out, 2)):
                for b in range(NCORES):
                    src = v_out[b * V_H:(b + 1) * V_H].rearrange(
                        "(p s m) -> p s m", p=128, m=DV)
                    nc.gpsimd.dma_start(
                        v_sb[:, b * ST + soff:b * ST + soff + 2, :], src)

            expS = expp.tile([128, JT, ROWS], BF16)

            # ---------------- S phase -------------------------------------
            with (
                tc.tile_pool(name="ktg", bufs=4) as ktg_pool,
                tc.tile_pool(name="ps_s", bufs=4, space="PSUM") as ps_s,
                tc.tile_pool(name="ps_rs", bufs=1, space="PSUM") as ps_rs,
            ):
                rs_ps = ps_rs.tile([128, 512], F32)
                n_rs = 0
                for half, kt_out_h in ((0, kt1_out), (1, kt2_out)):
                    for b in range(NCORES):
                        ktb = ktg_pool.tile([128, MT, HN], F16, tag="ktg")
                        src = kt_out_h[b * KT_H:(b + 1) * KT_H].rearrange(
                            "(p m n) -> p m n", p=128, n=HN).bitcast(F16)
                        nc.sync.dma_start(ktb[:, 0:4, :], src[:, 0:4, :])
                        nc.sync.dma_start(ktb[:, 4:8, :], src[:, 4:8, :])
                        for jj in range(2):
                            j = b * ST + half * 2 + jj
                            ps_S = ps_s.tile([128, 512], F32, tag="pss")
                            for t in range(MT):
                                nc.tensor.matmul(
                                    ps_S[:],
                                    ktb[:, t, jj * 128:(jj + 1) * 128],
                                    qt_sb[:, t, :],
                                    start=(t == 0), stop=(t == MT - 1))
                            nc.scalar.activation(expS[:, j, :], ps_S[:], Exp,
                                                 bias=bias_sb[:], scale=1.0)
                            nc.tensor.matmul(rs_ps[:], ones128[:],
                                             expS[:, j, :],
                                             start=(n_rs == 0),
                                             stop=(n_rs == JT - 1))
                            n_rs += 1

                # row-sum -> per-row reciprocal multipliers [128, ST]
                rs_sb = misc.tile([1, 512], F32)
                nc.vector.tensor_copy(rs_sb[:], rs_ps[0:1, :])
                rs_dram = dram.tile([1, 512], F32)
                nc.sync.dma_start(rs_dram[:], rs_sb[:])
                rs128 = misc.tile([128, ST], F32)
                nc.sync.dma_start(
                    rs128[:], rs_dram[0, :].rearrange("(r p) -> p r", p=128))
                mult_sb = misc.tile([128, ST], F32)
                nc.vector.reciprocal(mult_sb[:], rs128[:])
                nc.vector.tensor_scalar_mul(mult_sb[:], mult_sb[:], SCALE)

            # ---------------- AV phase ------------------------------------
            # Two accumulation passes over all 8 (h, r) PSUM banks:
            # pass 0 consumes j%4 in {0,1} (vA), pass 1 j%4 in {2,3} (vB) —
            # so vB may land up to ~30us into the AV phase without stalling.
            j_pass = ([j for j in range(JT) if j % 4 < 2],
                      [j for j in range(JT) if j % 4 >= 2])
            with tc.tile_pool(name="ps_o", bufs=8, space="PSUM") as ps_o:
                po = [ps_o.tile([128, 512], F32, tag="pso", name=f"po{g}")
                      for g in range(8)]
                for part in (0, 1):
                    for h in range(2):
                        for r in range(ST):
                            p = po[h * ST + r]
                            for idx, j in enumerate(j_pass[part]):
                                nc.tensor.matmul(
                                    p[:],
                                    expS[:, j, r * 128:(r + 1) * 128],
                                    v_sb[:, j, h * 512:(h + 1) * 512],
                                    start=(part == 0 and idx == 0),
                                    stop=(part == 1 and idx == len(j_pass[1]) - 1))
                            if part == 1:
                                o_sb = outp.tile([128, 512], F32, tag="osb")
                                nc.vector.tensor_scalar_mul(o_sb[:], p[:],
                                                            mult_sb[:, r:r + 1])
                                nc.sync.dma_start(
                                    d_out[r * 128:(r + 1) * 128,
                                          h * 512:(h + 1) * 512],
                                    o_sb[:])
    nc.compile()
    return nc


_BUILT = None


def kernel(z, Wq, Wk, Wv):
    global _BUILT
    from concourse.bass_utils import run_bass_kernel_spmd

    if _BUILT is None:
        _BUILT = _build()
    nc = _BUILT

    zT = np.ascontiguousarray(z.T).astype(np.float16)
    wq16 = Wq.astype(np.float16)
    wk16 = Wk.astype(np.float16)
    wv16 = Wv.astype(np.float16)
    in_maps = [
        {
            "zT": np.ascontiguousarray(zT[:, c * ROWS:(c + 1) * ROWS]),
            "Wq": wq16,
            "Wk": wk16,
            "Wv": wv16,
        }
        for c in range(NCORES)
    ]
    res = run_bass_kernel_spmd(nc, in_maps, list(range(NCORES)))
    out = np.concatenate([res.results[c]["out"] for c in range(NCORES)], axis=0)
    return out.astype(np.float32)


if __name__ == "__main__":
    rng = np.random.default_rng(0)
    z = rng.standard_normal((SEQ, D)).astype(np.float32)
    Wq = (0.02 * rng.standard_normal((D, DK))).astype(np.float32)
    Wk = (0.02 * rng.standard_normal((D, DK))).astype(np.float32)
    Wv = (0.02 * rng.standard_normal((D, DV))).astype(np.float32)
    out = kernel(z=z, Wq=Wq, Wk=Wk, Wv=Wv)
    print(out.shape, out.dtype)


# revision 19
# speedup vs baseline: 1.0669x; 1.0177x over previous
"""Distributed single-head attention on 8 TRN2 NeuronCores.

Math (matches the reference):
    q = z @ Wq; k = z @ Wk; v = z @ Wv
    out = softmax(q k^T) * DK**-0.5 @ v

Sharding: z rows split 8 ways. Each core projects its own shard; K^T
(fp16) and V (bf16) shards are all-gathered in four async halves,
chained in consumption order kt1 -> vA -> kt2 -> vB on the serialized
CC stream. Flash-style row-block attention follows:
    S^T_j = K^T[:, j-tile] ^T-matmul Q^T           (fp16 operands, f32 PSUM)
    P_j   = exp(S^T_j - 40)                        (bf16, shift-invariant)
    rowsumT = ones^T @ P                           (PE)
    out   = (P^T-matmuls V) * (scale / rowsum)

Schedule notes (all measured on this part):
 - cross-core launch skew is ~25-30us and each AllGather costs
   ~22-30us wall on the ONE serialized CC stream, so the chain is
   co-critical with the PE. V is projected between the two K halves so
   the chain order becomes kt1, vA, kt2, vB (trigger FIFO on the
   gpsimd queue enforces it even though vB's data is staged early).
 - the AV phase is split by output half AND by j-parity into four
   segments: AV(h0, j%4<2) runs BETWEEN the S halves — it needs only
   S-half0's P tiles plus vA, and fills the PE stall while kt2 is in
   flight. Bank budget: 2 (S) + 1 (rowsum) + 4 (AV h0) = 7 of 8.
 - input HBM BW is ~250-300GB/s shared, so projections are t-outer
   across all 8 PSUM banks (first matmul needs only the first chunk
   pair); input loads are strictly need-ordered on the sync ring;
   staging DMAs ride the idle ACT ring; V loads ride SWDGE.
 - bounce buffers are (p, m, n)-contiguous per partition; each gather
   half is staged with ONE dma.

Precision: fp16 z/W/Q/K + f32 PSUM keeps logits to ~1e-2 abs err;
exp/V/AV in bf16. End-to-end rel err ~3e-3 (vs f32 reference).
"""

import numpy as np

SEQ, D, DK, DV = 4096, 1024, 1024, 1024
NCORES = 8
ROWS = SEQ // NCORES            # 512 rows per core
DT = D // 128                   # 8 contraction tiles (input dim)
MT = DK // 128                  # 8 dk tiles
ST = ROWS // 128                # 4 local seq tiles
JT = SEQ // 128                 # 32 global seq tiles
HN = ROWS // 2                  # 256 = half the local rows
SHIFT = 40.0                    # constant logit shift (softmax-invariant)
SCALE = DK ** -0.5

KT_H = DK * HN                  # elems in one K^T half (bf16-bitcast fp16)
V_H = HN * DV                   # elems in one V half (bf16)


def _build():
    import concourse.mybir as mybir
    import concourse.tile as tile
    from concourse import bacc

    F32 = mybir.dt.float32
    F16 = mybir.dt.float16
    BF16 = mybir.dt.bfloat16
    Exp = mybir.ActivationFunctionType.Exp

    nc = bacc.Bacc("TRN2", target_bir_lowering=False, debug=False, num_devices=NCORES)
    d_zT = nc.declare_dram_parameter("zT", [D, ROWS], F16, isOutput=False)
    d_wq = nc.declare_dram_parameter("Wq", [D, DK], F16, isOutput=False)
    d_wk = nc.declare_dram_parameter("Wk", [D, DK], F16, isOutput=False)
    d_wv = nc.declare_dram_parameter("Wv", [D, DV], F16, isOutput=False)
    d_out = nc.declare_dram_parameter("out", [ROWS, DV], F32, isOutput=True)

    rg = [list(range(NCORES))]

    with tile.TileContext(nc) as tc:
        with (
            tc.tile_pool(name="dram", bufs=1, space="DRAM") as dram,
            tc.tile_pool(name="qt", bufs=1) as qt_pool,
            tc.tile_pool(name="misc", bufs=1) as misc,
            tc.tile_pool(name="stage", bufs=2) as stage,
            tc.tile_pool(name="vg", bufs=1) as vg_pool,
            tc.tile_pool(name="expp", bufs=1) as expp,
            tc.tile_pool(name="outp", bufs=2) as outp,
        ):
            # ---- collective bounce buffers (per local-seq half) ----
            kt1_in = dram.tile([KT_H], BF16)
            kt1_out = dram.tile([NCORES * KT_H], BF16, addr_space="Shared")
            kt2_in = dram.tile([KT_H], BF16)
            kt2_out = dram.tile([NCORES * KT_H], BF16, addr_space="Shared")
            va_in = dram.tile([V_H], BF16)
            va_out = dram.tile([NCORES * V_H], BF16, addr_space="Shared")
            vb_in = dram.tile([V_H], BF16)
            vb_out = dram.tile([NCORES * V_H], BF16, addr_space="Shared")

            # constants: full-width ones for the PE rowsum (M=128 runs at
            # standard N=512 rate; M=1 measured ~40% slower), exp bias
            ones128 = misc.tile([128, 128], BF16)
            nc.vector.memset(ones128[:], 1.0)
            bias_sb = misc.tile([128, 1], F32)
            nc.vector.memset(bias_sb[:], -SHIFT)
            # touch Exp once so the ACT table set loads before the S phase
            warm_sb = misc.tile([128, 1], F32)
            nc.scalar.activation(warm_sb[:], bias_sb[:], Exp,
                                 bias=bias_sb[:], scale=1.0)

            # ---------------- projection phase (scoped weights) ----------
            with (
                tc.tile_pool(name="wz", bufs=1) as wz,
                tc.tile_pool(name="ps_proj", bufs=8, space="PSUM") as ps_proj,
            ):
                # zT + Wk chunk pairs interleaved on the sync HWDGE ring so
                # the t-outer K projection starts after one pair lands
                zv = d_zT.rearrange("(t p) n -> p t n", p=128)
                wkv = d_wk.rearrange("(t p) m -> p t m", p=128)
                zT_sb, wk_sb = [], []
                for t in range(DT):
                    zt = wz.tile([128, ROWS], F16, name=f"zt{t}")
                    nc.sync.dma_start(zt[:], zv[:, t, :])
                    zT_sb.append(zt)
                    w = wz.tile([128, DK], F16, name=f"wk{t}")
                    nc.sync.dma_start(w[:], wkv[:, t, :])
                    wk_sb.append(w)
                # Wv/Wq behind zT/Wk on the SAME sync ring: strict FIFO
                # defers them so the K-critical 3MB gets full HBM bandwidth
                wv_sb, wq_sb = [], []
                for d_w, prefix, tiles in ((d_wv, "wv", wv_sb), (d_wq, "wq", wq_sb)):
                    wvw = d_w.rearrange("(t p) m -> p t m", p=128)
                    for t in range(DT):
                        w = wz.tile([128, DK], F16, name=f"{prefix}{t}")
                        nc.sync.dma_start(w[:], wvw[:, t, :])
                        tiles.append(w)

                def k_half(half, kt_in, kt_out):
                    psk = [ps_proj.tile([128, HN], F32, tag="psproj",
                                        name=f"psk{half}{m}") for m in range(MT)]
                    for t in range(DT):
                        for m in range(MT):
                            nc.tensor.matmul(
                                psk[m][:], wk_sb[t][:, m * 128:(m + 1) * 128],
                                zT_sb[t][:, half * HN:(half + 1) * HN],
                                start=(t == 0), stop=(t == DT - 1))
                    kt_stage = stage.tile([128, MT, HN], F16, tag="ktstage")
                    for m in range(MT):
                        nc.vector.tensor_copy(kt_stage[:, m, :], psk[m][:])
                    nc.scalar.dma_start(
                        kt_in[:].rearrange("(p m n) -> p m n", p=128, n=HN),
                        kt_stage[:].bitcast(BF16))
                    nc.gpsimd.collective_compute(
                        "AllGather", mybir.AluOpType.bypass, replica_groups=rg,
                        ins=[kt_in[:].opt()], outs=[kt_out[:].opt()])

                # K^T first seq-half: earliest trigger on the CC chain
                k_half(0, kt1_in, kt1_out)

                # V next (not K half2): its vA gather is the SECOND link of
                # the CC chain, feeding the AV(h0) segment that runs
                # between the S halves
                psv = [ps_proj.tile([128, 512], F32, tag="psproj",
                                    name=f"psv{g}") for g in range(8)]
                for t in range(DT):
                    for s in range(ST):
                        for h in range(2):
                            nc.tensor.matmul(
                                psv[s * 2 + h][:],
                                zT_sb[t][:, s * 128:(s + 1) * 128],
                                wv_sb[t][:, h * 512:(h + 1) * 512],
                                start=(t == 0), stop=(t == DT - 1))
                v_stages = []
                for v_in, s0 in ((va_in, 0), (vb_in, 2)):
                    v_stage = stage.tile([128, 2, DV], BF16, tag="vstage")
                    for s in (s0, s0 + 1):
                        for h in range(2):
                            nc.vector.tensor_copy(
                                v_stage[:, s - s0, h * 512:(h + 1) * 512],
                                psv[s * 2 + h][:])
                    nc.scalar.dma_start(
                        v_in[:].rearrange("(p s m) -> p s m", p=128, m=DV),
                        v_stage[:])
                nc.gpsimd.collective_compute(
                    "AllGather", mybir.AluOpType.bypass, replica_groups=rg,
                    ins=[va_in[:].opt()], outs=[va_out[:].opt()])

                # K^T second seq-half, then vB: gpsimd trigger FIFO makes
                # the CC chain process kt2 before vB even though vB's data
                # was staged during the V projection
                k_half(1, kt2_in, kt2_out)
                nc.gpsimd.collective_compute(
                    "AllGather", mybir.AluOpType.bypass, replica_groups=rg,
                    ins=[vb_in[:].opt()], outs=[vb_out[:].opt()])

                # Q^T: [DK, ROWS] fp16 resident, t-outer (tolerates the
                # staggered wq arrival behind wv on the sync ring)
                qt_sb = qt_pool.tile([128, MT, ROWS], F16)
                psq = [ps_proj.tile([128, 512], F32, tag="psproj",
                                    name=f"psq{m}") for m in range(MT)]
                for t in range(DT):
                    for m in range(MT):
                        nc.tensor.matmul(
                            psq[m][:], wq_sb[t][:, m * 128:(m + 1) * 128],
                            zT_sb[t][:],
                            start=(t == 0), stop=(t == DT - 1))
                for m in range(MT):
                    nc.vector.tensor_copy(qt_sb[:, m, :], psq[m][:])

            # V gathered: resident [128, JT, DV] bf16 (64KB/partition).
            # SWDGE loads, issued in (vA, vB) = consumption order.
            v_sb = vg_pool.tile([128, JT, DV], BF16)
            for v_out, soff in ((va_out, 0), (vb_out, 2)):
                for b in range(NCORES):
                    src = v_out[b * V_H:(b + 1) * V_H].rearrange(
                        "(p s m) -> p s m", p=128, m=DV)
                    nc.gpsimd.dma_start(
                        v_sb[:, b * ST + soff:b * ST + soff + 2, :], src)

            expS = expp.tile([128, JT, ROWS], BF16)

            # ---------------- S + AV phases -------------------------------
            # AV is split by output half h AND j-parity: (h0, j%4<2) runs
            # between the S halves; the rest after S. Each po group spans
            # both its passes (start on first p0 matmul, stop on last p1).
            j_pass = ([j for j in range(JT) if j % 4 < 2],
                      [j for j in range(JT) if j % 4 >= 2])

            def av_segment(po_h, h, part):
                for r in range(ST):
                    p = po_h[r]
                    for idx, j in enumerate(j_pass[part]):
                        nc.tensor.matmul(
                            p[:],
                            expS[:, j, r * 128:(r + 1) * 128],
                            v_sb[:, j, h * 512:(h + 1) * 512],
                            start=(part == 0 and idx == 0),
                            stop=(part == 1 and idx == len(j_pass[1]) - 1))
                    if part == 1:
                        o_sb = outp.tile([128, 512], F32, tag="osb")
                        nc.vector.tensor_scalar_mul(o_sb[:], p[:],
                                                    mult_sb[:, r:r + 1])
                        nc.sync.dma_start(
                            d_out[r * 128:(r + 1) * 128,
                                  h * 512:(h + 1) * 512],
                            o_sb[:])

            with tc.tile_pool(name="ps_oh0", bufs=4, space="PSUM") as ps_oh0:
                po_h0 = [ps_oh0.tile([128, 512], F32, tag="poh0",
                                     name=f"poh0{r}") for r in range(ST)]
                with (
                    tc.tile_pool(name="ktg", bufs=4) as ktg_pool,
                    tc.tile_pool(name="ps_s", bufs=2, space="PSUM") as ps_s,
                    tc.tile_pool(name="ps_rs", bufs=1, space="PSUM") as ps_rs,
                ):
                    rs_ps = ps_rs.tile([128, 512], F32)
                    n_rs = 0

                    def s_half(half, kt_out_h):
                        nonlocal n_rs
                        for b in range(NCORES):
                            ktb = ktg_pool.tile([128, MT, HN], F16, tag="ktg")
                            src = kt_out_h[b * KT_H:(b + 1) * KT_H].rearrange(
                                "(p m n) -> p m n", p=128, n=HN).bitcast(F16)
                            nc.sync.dma_start(ktb[:, 0:4, :], src[:, 0:4, :])
                            nc.sync.dma_start(ktb[:, 4:8, :], src[:, 4:8, :])
                            for jj in range(2):
                                j = b * ST + half * 2 + jj
                                ps_S = ps_s.tile([128, 512], F32, tag="pss")
                                for t in range(MT):
                                    nc.tensor.matmul(
                                        ps_S[:],
                                        ktb[:, t, jj * 128:(jj + 1) * 128],
                                        qt_sb[:, t, :],
                                        start=(t == 0), stop=(t == MT - 1))
                                nc.scalar.activation(expS[:, j, :], ps_S[:],
                                                     Exp, bias=bias_sb[:],
                                                     scale=1.0)
                                nc.tensor.matmul(rs_ps[:], ones128[:],
                                                 expS[:, j, :],
                                                 start=(n_rs == 0),
                                                 stop=(n_rs == JT - 1))
                                n_rs += 1

                    s_half(0, kt1_out)
                    # AV(h0, pass0): needs only S-half0's P tiles + vA;
                    # fills the PE while kt2 is still in flight
                    av_segment(po_h0, 0, 0)
                    s_half(1, kt2_out)

                    # row-sum -> per-row reciprocal multipliers [128, ST]
                    rs_sb = misc.tile([1, 512], F32)
                    nc.vector.tensor_copy(rs_sb[:], rs_ps[0:1, :])
                    rs_dram = dram.tile([1, 512], F32)
                    nc.sync.dma_start(rs_dram[:], rs_sb[:])
                    rs128 = misc.tile([128, ST], F32)
                    nc.sync.dma_start(
                        rs128[:], rs_dram[0, :].rearrange("(r p) -> p r",
                                                          p=128))
                    mult_sb = misc.tile([128, ST], F32)
                    nc.vector.reciprocal(mult_sb[:], rs128[:])
                    nc.vector.tensor_scalar_mul(mult_sb[:], mult_sb[:], SCALE)

                with tc.tile_pool(name="ps_oh1", bufs=4, space="PSUM") as ps_oh1:
                    po_h1 = [ps_oh1.tile([128, 512], F32, tag="poh1",
                                         name=f"poh1{r}") for r in range(ST)]
                    av_segment(po_h1, 1, 0)
                    av_segment(po_h0, 0, 1)
                    av_segment(po_h1, 1, 1)
    nc.compile()
    return nc


_BUILT = None


def kernel(z, Wq, Wk, Wv):
    global _BUILT
    from concourse.bass_utils import run_bass_kernel_spmd

    if _BUILT is None:
        _BUILT = _build()
    nc = _BUILT

    zT = np.ascontiguousarray(z.T).astype(np.float16)
    wq16 = Wq.astype(np.float16)
    wk16 = Wk.astype(np.float16)
    wv16 = Wv.astype(np.float16)
    in_maps = [
        {
            "zT": np.ascontiguousarray(zT[:, c * ROWS:(c + 1) * ROWS]),
            "Wq": wq16,
            "Wk": wk16,
            "Wv": wv16,
        }
        for c in range(NCORES)
    ]
    res = run_bass_kernel_spmd(nc, in_maps, list(range(NCORES)))
    out = np.concatenate([res.results[c]["out"] for c in range(NCORES)], axis=0)
    return out.astype(np.float32)


if __name__ == "__main__":
    rng = np.random.default_rng(0)
    z = rng.standard_normal((SEQ, D)).astype(np.float32)
    Wq = (0.02 * rng.standard_normal((D, DK))).astype(np.float32)
    Wk = (0.02 * rng.standard_normal((D, DK))).astype(np.float32)
    Wv = (0.02 * rng.standard_normal((D, DV))).astype(np.float32)
    out = kernel(z=z, Wq=Wq, Wk=Wk, Wv=Wv)
    print(out.shape, out.dtype)
